# revision 13
# baseline (speedup 1.0000x reference)
"""Trainium2 Bass kernel for DifferentiableRankIntegration.

Math (per query row i, B=1024, tau=0.1, K=60):
  Sp[j]  = sum_k pos[i,k]*sigmoid((s[i,k]-s[i,j])/tau)
  Sa[j]  = sum_k sigmoid((s[i,k]-s[i,j])/tau)
  rank[j]= 1 + pos[i,j]*Sa[j] + (1-2*pos[i,j])*Sp[j]
  out    = (K+1)*(w_v/(K+rank_v) + w_l/(K+rank_l))

Algorithm (Fourier factorization; O(N*B^2) instead of O(B^3)):
  With x=clip(s,-a,a):  sigmoid(u/tau) ~ 1/2 + gamma*u + sum_n b_n sin(w_n u).
  sin(w_n(x_k-x_j)) separates, so per row only masked feature sums over k
  (A_n = sum_k m_k sin(w_n x_k), B_n = sum_k m_k cos(w_n x_k)) are needed;
  S^m[j] = affine + sum_n b_n[A_n cos(w_n x_j) - B_n sin(w_n x_j)].
  B_n is centered (B~ = B_n - mu_n*M, mu_n ~ E cos(w_n X)) so every bf16
  feature tile multiplies an O(sqrt(B)) coefficient; the extracted
  mean-field term M*G(x_j), G = sum_n b_n mu_n sin(w_n .), is evaluated
  in fp32 via a fit on {tanh(beta y), y, sin(w_1..3 y)} whose sin parts
  fold back into the per-term coefficients (mu'_n = mu_n - g_n/b_n).

  The ScalarE Sin activation only covers args in [-pi, pi], so only
  sin/cos at w_1 = pi/(2a') are computed directly; higher harmonics use
  the Chebyshev ladder  f_n = 2cos_1 .* f_{n-1} - f_{n-2}  on DVE in
  bf16 (2x mode), with the scalar_tensor_tensor accum_out providing the
  unmasked row sums for free.

Engine mapping per core (128 rows in partitions, 8 cores row-parallel):
  ScalarE: sin_1/cos_1/tanh activations, Copy accums for T = sum x,
           Identity act for the per-row affine part.
  DVE:     harmonic ladder, tensor_tensor_reduce (pos .* feat -> masked
           accums, bf16 2x), per-n coefficient fixups, diag lhsT builds,
           finals.
  TensorE: per-term matmul with diagonal lhsT accumulating
           S^m in PSUM: out[c,j] += coef_n[c]*feat_n[c,j].
"""

import numpy as np

B = 1024
NCORES = 8
ROWS = B // NCORES  # 128 rows per core
P = 128
TAU = 0.1
K = 60.0

HARMONICS = [1, 2, 3, 4, 6]  # harmonics of pi/(2*A_HALF); 5 adds nothing
# Chebyshev ladder: (harmonic, parent1, parent2, doubler) with
# f_n = 2*cos_step .* f_p1 - f_p2;  't2c' doubles by 1, 't2c2' by 2.
LADDER = [(2, 1, 0, "t2c"), (3, 2, 1, "t2c"), (4, 3, 2, "t2c"),
          (6, 4, 2, "t2c2")]
N_F = len(HARMONICS)
N_G = 3          # ladder sins used in the G (mean-field) fit
A_CLAMP = 3.5
A_HALF = 3.52    # half-period of the sin basis (> A_CLAMP for headroom)


def _fit():
    U = 2 * A_HALF
    u = np.linspace(-2 * A_CLAMP, 2 * A_CLAMP, 16001)
    target = 1.0 / (1.0 + np.exp(-u / TAU)) - 0.5
    w = np.exp(-(u ** 2) / 4.0) + 0.02
    om = np.array([n * np.pi / U for n in HARMONICS])
    X = np.stack([u] + [np.sin(o * u) for o in om], 1)
    Ws = np.sqrt(w)[:, None]
    coef, *_ = np.linalg.lstsq(X * Ws, target * Ws[:, 0], rcond=None)
    gamma, b = float(coef[0]), coef[1:]

    mu = np.exp(-om ** 2 / 2.0)
    y = np.linspace(-A_CLAMP, A_CLAMP, 8001)
    G = np.sin(np.outer(y, om)) @ (b * mu)
    best = None
    for beta in np.linspace(0.4, 0.88, 25):
        Xg = np.stack(
            [np.tanh(beta * y), y] + [np.sin(om[i] * y) for i in range(N_G)], 1
        )
        cg, *_ = np.linalg.lstsq(Xg, G, rcond=None)
        mx = np.abs(G - Xg @ cg).max()
        if best is None or mx < best[0]:
            best = (mx, beta, cg)
    _, beta, cg = best
    alpha, rho, g = float(cg[0]), float(cg[1]), cg[2:]
    # fold G's sin corrections into the centering constants
    mup = mu.copy()
    for i in range(N_G):
        mup[i] = mu[i] - g[i] / b[i]
    return om, gamma, b, mup, beta, alpha, rho


OM, GAMMA, BN, MUP, BETA, ALPHA, RHO = _fit()


def _build_bass():
    import concourse.bacc as bacc
    import concourse.mybir as mybir
    from concourse.tile import TileContext

    f32 = mybir.dt.float32
    bf16 = mybir.dt.bfloat16
    Sin = mybir.ActivationFunctionType.Sin
    Tanh = mybir.ActivationFunctionType.Tanh
    Copy = mybir.ActivationFunctionType.Copy
    Ident = mybir.ActivationFunctionType.Identity
    mult = mybir.AluOpType.mult
    add = mybir.AluOpType.add
    subtract = mybir.AluOpType.subtract
    HALF = 512

    nc = bacc.Bacc()

    # per-core inputs (host pre-sharded; s pre-clamped to [-a, a])
    sv = nc.declare_dram_parameter("sv", [ROWS, B], f32, isOutput=False)
    sl = nc.declare_dram_parameter("sl", [ROWS, B], f32, isOutput=False)
    spv = nc.declare_dram_parameter("spv", [ROWS, B], f32, isOutput=False)
    spl = nc.declare_dram_parameter("spl", [ROWS, B], f32, isOutput=False)
    posf = nc.declare_dram_parameter("posf", [ROWS, B], f32, isOutput=False)
    posb = nc.declare_dram_parameter("posb", [ROWS, B], bf16, isOutput=False)
    w61v = nc.declare_dram_parameter("w61v", [ROWS, B], f32, isOutput=False)
    w61l = nc.declare_dram_parameter("w61l", [ROWS, B], f32, isOutput=False)
    # constants: b_n-scaled identity blocks for diag builds
    bigI = nc.declare_dram_parameter("bigI", [P, N_F * P], bf16, isOutput=False)
    out = nc.declare_dram_parameter("out", [ROWS, B], f32, isOutput=True)

    with TileContext(nc) as tc:
        with (
            tc.tile_pool(name="const", bufs=1) as cpool,
            tc.tile_pool(name="data", bufs=2) as dpool,
            tc.tile_pool(name="feat", bufs=28) as fpool,
            tc.tile_pool(name="scr", bufs=4) as spool,
            tc.tile_pool(name="diag", bufs=10) as gpool,
            tc.tile_pool(name="cols", bufs=1) as lpool,
            tc.tile_pool(name="fin", bufs=2) as npool,
            tc.tile_pool(name="psum", bufs=1, space="PSUM") as ppool,
        ):
            # --- constants ---
            bigI_t = cpool.tile([P, N_F * P], bf16, tag="bigI")
            nc.sync.dma_start(out=bigI_t[:], in_=bigI[:])
            posf_t = cpool.tile([ROWS, B], f32, tag="posf")
            posb_t = cpool.tile([ROWS, B], bf16, tag="posb")
            nc.sync.dma_start(out=posf_t[:], in_=posf[:])
            nc.sync.dma_start(out=posb_t[:], in_=posb[:])
            zero_t = cpool.tile([ROWS, B], bf16, tag="zero")
            one_t = cpool.tile([ROWS, B], bf16, tag="one")
            nc.gpsimd.memset(zero_t[:], 0.0)
            nc.gpsimd.memset(one_t[:], 1.0)

            # Mp = rowsum(pos) and per-row scale cols shared by both sims
            mcols = lpool.tile([P, 8], f32, tag="mcols")
            Mp = mcols[:, 0:1]
            halfMp = mcols[:, 1:2]
            sclin_p = mcols[:, 2:3]
            sctanh_p = mcols[:, 3:4]
            pio2 = mcols[:, 4:5]
            nc.gpsimd.memset(pio2, float(np.pi / 2))
            nc.vector.tensor_reduce(
                out=Mp, in_=posf_t[:], axis=mybir.AxisListType.X, op=add
            )
            nc.vector.tensor_scalar_mul(halfMp, Mp, 0.5)
            nc.vector.tensor_scalar_mul(sclin_p, Mp, -(GAMMA + RHO))
            nc.vector.tensor_scalar_mul(sctanh_p, Mp, -ALPHA)

            # one PSUM tile: 4 regions x [128,1024] fp32 (2 banks each)
            acc = ppool.tile([P, 4 * B], f32, tag="acc")

            # Sp/Sn destination tiles (filled row by row)
            res_sims = []
            for si, (s_in, sp_in, w_in) in enumerate(
                ((sv, spv, w61v), (sl, spl, w61l))
            ):
                s_t = dpool.tile([ROWS, B], f32, tag="s")
                sp_t = dpool.tile([ROWS, B], f32, tag="sp")
                w_t = dpool.tile([ROWS, B], f32, tag="w")
                nc.sync.dma_start(out=s_t[:], in_=s_in[:])
                nc.sync.dma_start(out=sp_t[:], in_=sp_in[:])
                nc.sync.dma_start(out=w_t[:], in_=w_in[:])

                cols = lpool.tile([P, 6 * N_F + 8], f32, tag=f"cols{si}")
                cA_a = cols[:, 0 * N_F : 1 * N_F]      # A_all accums
                cB_a = cols[:, 1 * N_F : 2 * N_F]      # B_all raw accums
                cA_p = cols[:, 2 * N_F : 3 * N_F]      # A_pos accums
                cB_p = cols[:, 3 * N_F : 4 * N_F]      # B_pos raw accums
                cNB_a = cols[:, 4 * N_F : 5 * N_F]     # mu'_n*B - B_all
                cNB_p = cols[:, 5 * N_F : 6 * N_F]     # mu'_n*Mp - B_pos
                cT_a = cols[:, 6 * N_F : 6 * N_F + 1]
                cT_p = cols[:, 6 * N_F + 1 : 6 * N_F + 2]
                bias_a = cols[:, 6 * N_F + 2 : 6 * N_F + 3]
                bias_p = cols[:, 6 * N_F + 3 : 6 * N_F + 4]

                # PSUM region slices
                r_all = acc[:, (2 * si) * B : (2 * si + 1) * B]
                r_pos = acc[:, (2 * si + 1) * B : (2 * si + 2) * B]
                n_terms = 2 * N_F  # matmul terms per mask
                counts = {"all": 0, "pos": 0}

                def mm2(key, lhsT, rhs, counts=counts, r_all=r_all, r_pos=r_pos,
                        n_terms=n_terms):
                    region = r_all if key == "all" else r_pos
                    i = counts[key]
                    counts[key] = i + 1
                    for h in (0, 1):
                        nc.tensor.matmul(
                            out=region[:, h * HALF : (h + 1) * HALF],
                            lhsT=lhsT,
                            rhs=rhs[:, h * HALF : (h + 1) * HALF],
                            start=(i == 0),
                            stop=(i == n_terms - 1),
                        )

                # T accums via Copy activations (scratch out)
                scr = spool.tile([ROWS, B], bf16, tag="scr")
                nc.scalar.activation(
                    out=scr[:], in_=s_t[:], func=Copy, accum_out=cT_a
                )
                scr2 = spool.tile([ROWS, B], bf16, tag="scr")
                nc.scalar.activation(
                    out=scr2[:], in_=sp_t[:], func=Copy, accum_out=cT_p
                )
                # tanh feature (fp32)
                tanh_t = npool.tile([ROWS, B], f32, tag="tanh")
                nc.scalar.activation(out=tanh_t[:], in_=s_t[:], func=Tanh, scale=BETA)

                # affine bias cols: 0.5*M + gamma*T
                nc.vector.tensor_scalar(
                    out=bias_a, in0=cT_a, scalar1=GAMMA, scalar2=0.5 * B,
                    op0=mult, op1=add,
                )
                nc.vector.scalar_tensor_tensor(
                    out=bias_p, in0=cT_p, scalar=GAMMA, in1=halfMp,
                    op0=mult, op1=add,
                )

                # ---- base features on ScalarE (args within [-pi, pi]) ----
                w1 = float(np.pi / (2 * A_HALF))
                s1 = fpool.tile([ROWS, B], bf16, tag="f")
                nc.scalar.activation(
                    out=s1[:], in_=s_t[:], func=Sin, scale=w1,
                    accum_out=cA_a[:, 0:1],
                )
                c1 = fpool.tile([ROWS, B], bf16, tag="f")
                nc.scalar.activation(
                    out=c1[:], in_=s_t[:], func=Sin, scale=w1, bias=pio2,
                    accum_out=cB_a[:, 0:1],
                )
                H = {0: (zero_t, one_t), 1: (s1, c1)}
                t2c = fpool.tile([ROWS, B], bf16, tag="f")
                nc.vector.tensor_scalar_mul(t2c[:], c1[:], 2.0)
                doublers = {"t2c": t2c}

                def coef_terms(n, sin_n, cos_n):
                    """masked accums, fixups, diag builds, 8 matmuls for n."""
                    blkI = bigI_t[:, n * P : (n + 1) * P]
                    dA = gpool.tile([P, P], bf16, tag="d")
                    nc.vector.tensor_scalar_mul(dA[:], blkI, cA_a[:, n : n + 1])
                    mm2("all", dA[:], cos_n[:])
                    to1 = spool.tile([ROWS, B], bf16, tag="scr")
                    nc.vector.scalar_tensor_tensor(
                        out=to1[:], in0=sin_n[:], scalar=1.0, in1=posb_t[:],
                        op0=mult, op1=mult,
                        accum_out=cA_p[:, n : n + 1],
                    )
                    dAp = gpool.tile([P, P], bf16, tag="d")
                    nc.vector.tensor_scalar_mul(dAp[:], blkI, cA_p[:, n : n + 1])
                    mm2("pos", dAp[:], cos_n[:])
                    to2 = spool.tile([ROWS, B], bf16, tag="scr")
                    nc.vector.scalar_tensor_tensor(
                        out=to2[:], in0=cos_n[:], scalar=1.0, in1=posb_t[:],
                        op0=mult, op1=mult,
                        accum_out=cB_p[:, n : n + 1],
                    )
                    nc.vector.tensor_scalar(
                        out=cNB_a[:, n : n + 1], in0=cB_a[:, n : n + 1],
                        scalar1=-1.0, scalar2=float(B * MUP[n]),
                        op0=mult, op1=add,
                    )
                    dB = gpool.tile([P, P], bf16, tag="d")
                    nc.vector.tensor_scalar_mul(dB[:], blkI, cNB_a[:, n : n + 1])
                    mm2("all", dB[:], sin_n[:])
                    nc.vector.scalar_tensor_tensor(
                        out=cNB_p[:, n : n + 1], in0=Mp, scalar=float(MUP[n]),
                        in1=cB_p[:, n : n + 1], op0=mult, op1=subtract,
                    )
                    dBp = gpool.tile([P, P], bf16, tag="d")
                    nc.vector.tensor_scalar_mul(dBp[:], blkI, cNB_p[:, n : n + 1])
                    mm2("pos", dBp[:], sin_n[:])

                coef_terms(0, s1, c1)

                # ---- harmonic ladder on DVE: f_n = 2cos_d*f_p1 - f_p2 ----
                for (n, p1, p2, dk) in LADDER:
                    if dk not in doublers:
                        assert dk == "t2c2"
                        t2c2 = fpool.tile([ROWS, B], bf16, tag="f")
                        nc.vector.tensor_scalar_mul(t2c2[:], H[2][1][:], 2.0)
                        doublers[dk] = t2c2
                    dbl = doublers[dk]
                    idx = HARMONICS.index(n)
                    tmp_s = spool.tile([ROWS, B], bf16, tag="scr")
                    nc.vector.tensor_mul(tmp_s[:], dbl[:], H[p1][0][:])
                    sin_n = fpool.tile([ROWS, B], bf16, tag="f")
                    nc.vector.scalar_tensor_tensor(
                        out=sin_n[:], in0=tmp_s[:], scalar=1.0,
                        in1=H[p2][0][:], op0=mult, op1=subtract,
                        accum_out=cA_a[:, idx : idx + 1],
                    )
                    tmp_c = spool.tile([ROWS, B], bf16, tag="scr")
                    nc.vector.tensor_mul(tmp_c[:], dbl[:], H[p1][1][:])
                    cos_n = fpool.tile([ROWS, B], bf16, tag="f")
                    nc.vector.scalar_tensor_tensor(
                        out=cos_n[:], in0=tmp_c[:], scalar=1.0,
                        in1=H[p2][1][:], op0=mult, op1=subtract,
                        accum_out=cB_a[:, idx : idx + 1],
                    )
                    H[n] = (sin_n, cos_n)
                    coef_terms(idx, sin_n, cos_n)
                assert counts["all"] == counts["pos"] == n_terms

                # per-row affine part (late: bias cols are long since ready)
                t1_a = npool.tile([ROWS, B], f32, tag="t1a")
                t1_p = npool.tile([ROWS, B], f32, tag="t1p")
                nc.scalar.activation(
                    out=t1_a[:], in_=s_t[:], func=Ident,
                    bias=bias_a, scale=-(GAMMA + RHO) * B,
                )
                nc.scalar.activation(
                    out=t1_p[:], in_=s_t[:], func=Ident,
                    bias=bias_p, scale=sclin_p,
                )

                # ---- merge: u = PSUM + t1 + tanh*(-alpha*M) ----
                t2_a = npool.tile([ROWS, B], f32, tag="t2a")
                t2_p = npool.tile([ROWS, B], f32, tag="t2p")
                nc.vector.scalar_tensor_tensor(
                    out=t2_a[:], in0=tanh_t[:], scalar=-ALPHA * B, in1=t1_a[:],
                    op0=mult, op1=add,
                )
                nc.vector.scalar_tensor_tensor(
                    out=t2_p[:], in0=tanh_t[:], scalar=sctanh_p, in1=t1_p[:],
                    op0=mult, op1=add,
                )
                u_a = npool.tile([ROWS, B], f32, tag="ua")
                u_p = npool.tile([ROWS, B], f32, tag="up")
                nc.vector.tensor_add(u_a[:], t2_a[:], r_all)
                nc.vector.tensor_add(u_p[:], t2_p[:], r_pos)

                # ---- rank + w61/(61+rank) ----
                t = npool.tile([ROWS, B], f32, tag="t")
                nc.vector.scalar_tensor_tensor(
                    out=t[:], in0=u_p[:], scalar=-2.0, in1=u_a[:],
                    op0=mult, op1=add,
                )
                nc.vector.tensor_mul(t[:], t[:], posf_t[:])
                nc.vector.scalar_tensor_tensor(
                    out=t[:], in0=t[:], scalar=K + 1.0, in1=u_p[:],
                    op0=add, op1=add,
                )
                nc.vector.reciprocal(t[:], t[:])
                r_sim = npool.tile([ROWS, B], f32, tag="rsim")
                nc.vector.tensor_mul(r_sim[:], t[:], w_t[:])
                res_sims.append(r_sim)

            res = npool.tile([ROWS, B], f32, tag="resf")
            nc.vector.tensor_add(res[:], res_sims[0][:], res_sims[1][:])
            nc.sync.dma_start(out=out[:], in_=res[:])

    nc.compile()
    return nc


_NC_CACHE = None


def _get_nc():
    global _NC_CACHE
    if _NC_CACHE is None:
        _NC_CACHE = _build_bass()
    return _NC_CACHE


_CONST_CACHE = None


def _consts():
    import ml_dtypes

    ii = np.eye(P, dtype=np.float32)
    bigI = np.concatenate([b * ii for b in BN], 1)  # [P, N_F*P]
    return {"bigI": np.ascontiguousarray(bigI).astype(ml_dtypes.bfloat16)}


def _prep_core_inputs(s_v, s_l, pos_f, neg_f, w_v, w_l, core):
    import ml_dtypes

    global _CONST_CACHE
    if _CONST_CACHE is None:
        _CONST_CACHE = _consts()
    lo, hi = core * ROWS, (core + 1) * ROWS
    svc = np.clip(s_v[lo:hi], -A_CLAMP, A_CLAMP).astype(np.float32)
    slc = np.clip(s_l[lo:hi], -A_CLAMP, A_CLAMP).astype(np.float32)
    ps = np.ascontiguousarray(pos_f[lo:hi]).astype(np.float32)
    d = {
        "sv": np.ascontiguousarray(svc),
        "sl": np.ascontiguousarray(slc),
        "spv": np.ascontiguousarray(ps * svc),
        "spl": np.ascontiguousarray(ps * slc),
        "posf": ps,
        "posb": ps.astype(ml_dtypes.bfloat16),
        "w61v": np.ascontiguousarray((K + 1.0) * w_v[lo:hi]).astype(np.float32),
        "w61l": np.ascontiguousarray((K + 1.0) * w_l[lo:hi]).astype(np.float32),
    }
    d.update(_CONST_CACHE)
    return d


def _run(in_maps, trace=False):
    from concourse.bass_utils import run_bass_kernel_spmd

    nc = _get_nc()
    return run_bass_kernel_spmd(nc, in_maps, core_ids=list(range(NCORES)), trace=trace)


def kernel(s_v, s_l, pos_mask, neg_mask, w_v, w_l, _trace=False):
    pos_f = pos_mask.astype(np.float32)
    neg_f = neg_mask.astype(np.float32)
    in_maps = [
        _prep_core_inputs(s_v, s_l, pos_f, neg_f, w_v, w_l, core)
        for core in range(NCORES)
    ]
    res = _run(in_maps, trace=_trace)
    outs = [res.results[i]["out"] for i in range(NCORES)]
    full = np.concatenate(outs, axis=0).astype(np.float32)
    if _trace:
        return full, res
    return full
